# revision 36
# baseline (speedup 1.0000x reference)
"""Bayes-by-Backprop LSTM on 8 Trainium2 NeuronCores (Bass/Tile).

V3 strategy (data parallel, transposed recurrence, bf16):
  - Shard batch B=256 across 8 cores (32 rows each); eps/weights replicated.
  - Host-side layout only (transpose/reshape/concat + dtype casts, no math):
      * merged weight rows [x(64); bias(1); h(128)] = 193, columns (gate, h)
        gate order (i, f, ch, o); ch columns later doubled on device so all
        four gates go through one Sigmoid (tanh(x) = 2*sigmoid(2x)-1).
      * eps_w/eps_b merged the same way, laid out [193, S, 512] bf16.
      * x transposed to [64, S, B] bf16 with a ones-row (bias via matmul).
  - Device per step t (all transposed: batch is the matmul MOVING dim N=32):
      w1 = sig1*eps1[t] + mu1 ; w2 = sig2*eps2[t] + mu2   (DVE stt, bf16 4x)
      per gate gi: P[:,gi,:] = w2[:,gi]^T @ xcT_t + w1[:,gi]^T @ hT
        (sampled weights are the PE STATIONARY [K,128]; moving N=32)
      s = sigmoid(P) [128, 4*32] -> bf16
      cT' = f*cT + i*tanh-part ; hT' = o*tanh(cT')  (DVE/Pool, [128,32] ops)
    hT' [128, 32] bf16 feeds the next step's matmul directly - no transpose.
  - Final: out = h_S @ (out_mu + softplus(out_rho)*eps_out) + bias via ones row.
"""

import os
import sys
import numpy as np

for _p in ("/opt/trn_rl_repo",):
    if _p not in sys.path and os.path.isdir(_p):
        sys.path.append(_p)

import ml_dtypes  # noqa: E402
import concourse.bass as bass  # noqa: E402
import concourse.tile as tile  # noqa: E402
from concourse import mybir  # noqa: E402
from concourse.bass_utils import run_bass_kernel_spmd  # noqa: E402

F32 = mybir.dt.float32
BF16 = mybir.dt.bfloat16
AF = mybir.ActivationFunctionType
ALU = mybir.AluOpType
N_CORES = 8
GPERM = [0, 1, 2, 3]  # gate column order: i, f, ch, o

# The walrus bundled in this container rejects instructions carrying more
# than 2 semaphore-wait commands; Tile freely emits 3+. Split the excess
# onto same-engine NOPs inserted immediately before the instruction.
MAX_INST_WAITS = 1


def _split_excess_waits(nc, max_waits=MAX_INST_WAITS):
    blocks = []
    for f in nc.m.functions:
        for blk in f.blocks:
            blocks.append((blk, list(blk.instructions)))

    plans = {}  # id(inst) -> list of nop mybir instructions to insert before
    for blk, insts in blocks:
        for inst in insts:
            si = inst.sync_info
            if si is None:
                continue
            waits = list(si.on_wait)
            if len(waits) <= max_waits:
                continue
            ge = [w for w in waits if w.wait_mode == "sem-ge-imm"]
            other = [w for w in waits if w.wait_mode != "sem-ge-imm"]
            assert len(other) <= max_waits, (
                f"{inst.name}: {len(other)} non-ge waits, cannot split"
            )
            keep_n = max_waits - len(other)
            kept = other + (ge[len(ge) - keep_n :] if keep_n > 0 else [])
            excess = ge[: len(ge) - keep_n] if keep_n > 0 else ge
            eng = inst.engine
            nops = []
            for k in range(0, len(excess), max_waits):
                nop = nc.engines[eng].nop()
                nop.ins.sync_info = mybir.SyncInfo(
                    on_wait=list(excess[k : k + max_waits]), on_update=[]
                )
                nops.append(nop.ins)
            inst.sync_info = mybir.SyncInfo(
                on_wait=kept, on_update=list(si.on_update)
            )
            plans[id(inst)] = nops

    if not plans:
        return
    for blk, orig in blocks:
        new = []
        for inst in orig:
            new.extend(plans.get(id(inst), ()))
            new.append(inst)
        blk.instructions = new


def _build_program(S, TB, shard, repeats=1):
    """Emit the bass program for one core (SPMD across 8)."""
    nc = bass.Bass()
    d_eps = nc.declare_dram_parameter("epsm", [193, S, 512], BF16, isOutput=False)
    d_xc = nc.declare_dram_parameter("xc", [65, S, shard], BF16, isOutput=False)
    d_mu = nc.declare_dram_parameter("mum", [193, 512], F32, isOutput=False)
    d_rho = nc.declare_dram_parameter("rhom", [193, 512], F32, isOutput=False)
    d_h0T = nc.declare_dram_parameter("h0T", [128, shard], F32, isOutput=False)
    d_c0T = nc.declare_dram_parameter("c0T", [128, shard], F32, isOutput=False)
    d_owm = nc.declare_dram_parameter("owm", [128, 8], F32, isOutput=False)
    d_owr = nc.declare_dram_parameter("owr", [128, 8], F32, isOutput=False)
    d_eow = nc.declare_dram_parameter("eow", [128, 8], F32, isOutput=False)
    d_obm = nc.declare_dram_parameter("obm", [1, 8], F32, isOutput=False)
    d_obr = nc.declare_dram_parameter("obr", [1, 8], F32, isOutput=False)
    d_eob = nc.declare_dram_parameter("eob", [1, 8], F32, isOutput=False)
    d_out = nc.declare_dram_parameter("out", [shard, 8], F32, isOutput=True)

    IL = int(os.environ.get("BASS_LSTM_IL", "2"))  # batch interleave chains
    assert shard % IL == 0
    hsh = shard // IL

    from contextlib import ExitStack

    with tile.TileContext(nc) as tc, ExitStack() as ctx:
        singles = ctx.enter_context(tc.tile_pool(name="singles", bufs=1))
        _nb = 3 if TB <= 8 else 2
        pe1 = ctx.enter_context(tc.tile_pool(name="pe1", bufs=_nb))
        pe2 = ctx.enter_context(tc.tile_pool(name="pe2", bufs=_nb))
        px = ctx.enter_context(tc.tile_pool(name="px", bufs=_nb))
        pw1 = ctx.enter_context(tc.tile_pool(name="pw1", bufs=2))
        pw2 = ctx.enter_context(tc.tile_pool(name="pw2", bufs=2))
        psm = ctx.enter_context(tc.tile_pool(name="psm", bufs=12 * IL))
        phT = ctx.enter_context(tc.tile_pool(name="phT", bufs=3 * IL))
        psum_g = ctx.enter_context(
            tc.tile_pool(name="psum_g", bufs=4, space=bass.MemorySpace.PSUM)
        )
        psum_o = ctx.enter_context(
            tc.tile_pool(name="psum_o", bufs=1, space=bass.MemorySpace.PSUM)
        )

        # ---- constants in SBUF
        mu1f = singles.tile([128, 512], F32)
        mu2f = singles.tile([65, 512], F32)
        nc.gpsimd.dma_start(mu1f[:], d_mu[65:193, :])
        nc.gpsimd.dma_start(mu2f[:], d_mu[0:65, :])
        rho1 = singles.tile([128, 512], F32)
        rho2 = singles.tile([65, 512], F32)
        nc.gpsimd.dma_start(rho1[:], d_rho[65:193, :])
        nc.gpsimd.dma_start(rho2[:], d_rho[0:65, :])

        # softplus(x) = ln(1 + exp(x)) - ln+exp share one table set
        def softplus(out_ap, in_ap, tmp_ap):
            nc.scalar.activation(tmp_ap, in_ap, AF.Exp)
            nc.vector.tensor_scalar_add(tmp_ap, tmp_ap, 1.0)
            nc.scalar.activation(out_ap, tmp_ap, AF.Ln)

        sig1f = singles.tile([128, 512], F32)
        sig2f = singles.tile([65, 512], F32)
        spt1 = singles.tile([128, 512], F32)
        spt2 = singles.tile([65, 512], F32)
        softplus(sig1f[:], rho1[:], spt1[:])
        softplus(sig2f[:], rho2[:], spt2[:])

        # ch-gate columns (256:384) pre-doubled: tanh(x) = 2*sigmoid(2x)-1 so
        # all four gates go through one Sigmoid instruction.
        for ap in (sig1f[:, 256:384], sig2f[:, 256:384],
                   mu1f[:, 256:384], mu2f[:, 256:384]):
            nc.vector.tensor_scalar_mul(ap, ap, 2.0)

        sig1 = singles.tile([128, 512], BF16)
        sig2 = singles.tile([65, 512], BF16)
        mu1 = singles.tile([128, 512], BF16)
        mu2 = singles.tile([65, 512], BF16)
        nc.vector.tensor_copy(sig1[:], sig1f[:])
        nc.vector.tensor_copy(sig2[:], sig2f[:])
        nc.vector.tensor_copy(mu1[:], mu1f[:])
        nc.vector.tensor_copy(mu2[:], mu2f[:])

        ones1f = singles.tile([1, shard], F32)
        nc.gpsimd.memset(ones1f[:], 1.0)
        ones1 = singles.tile([1, shard], BF16)
        nc.vector.tensor_copy(ones1[:], ones1f[:])

        # ---- output projection weights (tiny, f32 math then bf16)
        owm = singles.tile([128, 8], F32)
        owr = singles.tile([128, 8], F32)
        eow = singles.tile([128, 8], F32)
        nc.gpsimd.dma_start(owm[:], d_owm[:])
        nc.gpsimd.dma_start(owr[:], d_owr[:])
        nc.gpsimd.dma_start(eow[:], d_eow[:])
        sow = singles.tile([128, 8], F32)
        sowt = singles.tile([128, 8], F32)
        softplus(sow[:], owr[:], sowt[:])
        wtmp = singles.tile([128, 8], F32)
        nc.vector.tensor_mul(wtmp[:], sow[:], eow[:])
        woutf = singles.tile([128, 8], F32)
        nc.vector.tensor_add(woutf[:], wtmp[:], owm[:])
        wout = singles.tile([128, 8], BF16)
        nc.vector.tensor_copy(wout[:], woutf[:])

        obm = singles.tile([1, 8], F32)
        obr = singles.tile([1, 8], F32)
        eob = singles.tile([1, 8], F32)
        nc.gpsimd.dma_start(obm[:], d_obm[:])
        nc.gpsimd.dma_start(obr[:], d_obr[:])
        nc.gpsimd.dma_start(eob[:], d_eob[:])
        sob = singles.tile([1, 8], F32)
        sobt = singles.tile([1, 8], F32)
        softplus(sob[:], obr[:], sobt[:])
        btmp = singles.tile([1, 8], F32)
        nc.vector.tensor_mul(btmp[:], sob[:], eob[:])
        boutf = singles.tile([1, 8], F32)
        nc.vector.tensor_add(boutf[:], btmp[:], obm[:])
        bout = singles.tile([1, 8], BF16)
        nc.vector.tensor_copy(bout[:], boutf[:])

        # ---- replicate sig (and mu for the h-block fold) across TB for the
        # bulk per-block weight multiplies
        sig1r = singles.tile([128, TB, 512], BF16)
        sig2r = singles.tile([65, TB, 512], BF16)
        mu1r = singles.tile([128, TB, 512], BF16)
        for k in range(TB):
            nc.vector.tensor_copy(sig1r[:, k, :], sig1[:])
            nc.vector.tensor_copy(sig2r[:, k, :], sig2[:])
            nc.vector.tensor_copy(mu1r[:, k, :], mu1[:])

        # column split point between DVE and Pool for the weight multiplies
        C1 = int(os.environ.get("BASS_LSTM_C1", "384"))  # w1 (h-block) split
        C2 = int(os.environ.get("BASS_LSTM_C2", "256"))  # w2 (x-block) split

        # ---- state (per interleave chain): cT f32, hT bf16 (SBUF)
        cstall = singles.tile([128, IL, hsh], F32)
        h0f = singles.tile([128, shard], F32)
        nc.gpsimd.dma_start(h0f[:], d_h0T[:])
        assert S % TB == 0
        hT = [None] * IL
        for rep in range(repeats):
            for ch in range(IL):
                nc.gpsimd.dma_start(
                    cstall[:, ch, :], d_c0T[:, ch * hsh:(ch + 1) * hsh])
                h0b = phT.tile([128, hsh], BF16)
                nc.vector.tensor_copy(
                    h0b[:], h0f[:, ch * hsh:(ch + 1) * hsh])
                hT[ch] = h0b

            e1 = e2 = xb = w1 = w2 = None
            for t in range(S):
                tl = t % TB
                if tl == 0:
                    t0 = t
                    # DMA queues: keep the critical-cycle engines (ACT, Pool,
                    # PE) free. e1 -> SP; e2 split SP/DVE; xb -> SP.
                    e1 = pe1.tile([128, TB, 512], BF16)
                    nc.sync.dma_start(e1[:], d_eps[65:193, t0:t0 + TB, :])
                    e2 = pe2.tile([65, TB, 512], BF16)
                    nc.gpsimd.dma_start(e2[:], d_eps[0:65, t0:t0 + TB, :])
                    xb = px.tile([65, TB, shard], BF16)
                    nc.sync.dma_start(xb[:], d_xc[:, t0:t0 + TB, :])
                    # bulk sampled weights for the whole block; DVE and Pool
                    # are off the critical cycle. h-block fully folds mu so
                    # its mu matmul disappears (w1 = sig*eps + mu).
                    w1 = pw1.tile([128, TB, 512], BF16)
                    nc.vector.tensor_mul(w1[:], e1[:], sig1r[:])
                    nc.gpsimd.tensor_add(w1[:], w1[:], mu1r[:])
                    w2 = pw2.tile([65, TB, 512], BF16)
                    nc.vector.tensor_mul(w2[:], e2[:], sig2r[:])

                for ch in range(IL):
                    b0 = ch * hsh
                    g = psum_g.tile([128, 4, hsh], F32)
                    xlt = xb[:, tl, b0:b0 + hsh]
                    # one start->stop accumulation group per gate slice; a
                    # PSUM zero region admits only one pending group at a time
                    for gi in range(4):
                        c0_, c1_ = gi * 128, (gi + 1) * 128
                        nc.tensor.matmul(
                            g[:, gi, :], mu2[:, c0_:c1_], xlt,
                            start=True, stop=False)
                        nc.tensor.matmul(
                            g[:, gi, :], w2[:, tl, c0_:c1_], xlt,
                            start=False, stop=False)
                        nc.tensor.matmul(
                            g[:, gi, :], w1[:, tl, c0_:c1_], hT[ch][:],
                            start=False, stop=True)

                    s = psm.tile([128, 4, hsh], BF16)
                    nc.scalar.activation(s[:], g[:], AF.Sigmoid)

                    # i*tanh(g_ch)/2 = s_i*(s_ch'-0.5), s_ch' = sigmoid(2 g_ch)
                    # small recurrence ops consecutively on DVE: same-engine
                    # ordering avoids cross-engine hops on the critical cycle
                    v2 = psm.tile([128, hsh], BF16)
                    nc.vector.scalar_tensor_tensor(
                        v2[:], s[:, 2, :], -0.5, s[:, 0, :], ALU.add, ALU.mult)
                    fc = psm.tile([128, hsh], F32)
                    nc.gpsimd.tensor_mul(fc[:], s[:, 1, :], cstall[:, ch, :])
                    nc.vector.scalar_tensor_tensor(
                        cstall[:, ch, :], v2[:], 2.0, fc[:], ALU.mult, ALU.add)
                    th = psm.tile([128, hsh], BF16)
                    nc.scalar.activation(th[:], cstall[:, ch, :], AF.Tanh)
                    hnew = phT.tile([128, hsh], BF16)
                    nc.gpsimd.tensor_mul(hnew[:], s[:, 3, :], th[:])
                    hT[ch] = hnew

        # ---- output projection
        for ch in range(IL):
            b0 = ch * hsh
            ops = psum_o.tile([hsh, 8], F32, name=f"ops{ch}")
            nc.tensor.matmul(ops[:], hT[ch][:], wout[:],
                             start=True, stop=False)
            nc.tensor.matmul(ops[:], ones1[:, b0:b0 + hsh],
                             bout[:], start=False, stop=True)
            osb = psm.tile([hsh, 8], F32, name=f"osb{ch}")
            nc.vector.tensor_copy(osb[:], ops[:])
            nc.gpsimd.dma_start(d_out[b0:b0 + hsh, :], osb[:])

    predicted_ns = None
    try:
        ent = tc._perfetto_entries
        if ent:
            predicted_ns = int(max(max(e[1] or 0, e[2] or 0) for e in ent))
    except Exception:
        pass
    return nc, predicted_ns


def _host_layout(inputs):
    x = np.asarray(inputs["x"], np.float32)
    h0 = np.asarray(inputs["h0"], np.float32)
    c0 = np.asarray(inputs["c0"], np.float32)
    w_mu = np.asarray(inputs["w_mu"], np.float32)
    w_rho = np.asarray(inputs["w_rho"], np.float32)
    b_mu = np.asarray(inputs["b_mu"], np.float32)
    b_rho = np.asarray(inputs["b_rho"], np.float32)
    eps_w = np.asarray(inputs["eps_w"], np.float32)
    eps_b = np.asarray(inputs["eps_b"], np.float32)

    B, S, I = x.shape
    H = h0.shape[1]
    G = 4
    GH = G * H

    def merge_rows(w_g, b_g):  # w_g [G, I+H, H], b_g [G, H] -> [I+1+H, G*H]
        rows = np.transpose(w_g, (1, 0, 2)).reshape(I + H, GH)
        brow = b_g.reshape(1, GH)
        return np.concatenate([rows[:I], brow, rows[I:]], axis=0)

    mu_m = np.ascontiguousarray(merge_rows(w_mu[GPERM], b_mu[GPERM]))
    rho_m = np.ascontiguousarray(merge_rows(w_rho[GPERM], b_rho[GPERM]))

    ew = eps_w[:, GPERM]  # [S, G, I+H, H]
    eps_rows = np.transpose(ew, (0, 2, 1, 3)).reshape(S, I + H, GH)
    eb_row = eps_b[:, GPERM].reshape(S, 1, GH)
    eps_m = np.concatenate([eps_rows[:, :I], eb_row, eps_rows[:, I:]], axis=1)
    epsT = np.ascontiguousarray(
        np.transpose(eps_m, (1, 0, 2)).astype(ml_dtypes.bfloat16)
    )  # [193, S, GH] bf16

    xT = np.transpose(x, (2, 1, 0))  # [I, S, B]
    ones_row = np.ones((1, S, B), np.float32)
    xc_all = np.concatenate([xT, ones_row], axis=0).astype(
        ml_dtypes.bfloat16
    )  # [I+1, S, B] bf16

    h0T = np.ascontiguousarray(h0.T)  # [H, B]
    c0T = np.ascontiguousarray(c0.T)  # [H, B]

    ow_m = np.asarray(inputs["out_w_mu"], np.float32)
    ow_r = np.asarray(inputs["out_w_rho"], np.float32)
    e_ow = np.asarray(inputs["eps_out_w"], np.float32)
    ob_m = np.asarray(inputs["out_b_mu"], np.float32).reshape(1, -1)
    ob_r = np.asarray(inputs["out_b_rho"], np.float32).reshape(1, -1)
    e_ob = np.asarray(inputs["eps_out_b"], np.float32).reshape(1, -1)

    return dict(
        S=S, B=B, epsT=epsT, mu_m=mu_m, rho_m=rho_m, xc_all=xc_all,
        h0T=h0T, c0T=c0T, ow_m=ow_m, ow_r=ow_r, e_ow=e_ow,
        ob_m=ob_m, ob_r=ob_r, e_ob=e_ob,
    )


def prepare(_repeats=1, **inputs):
    """Build the bass program + per-core input maps."""
    L = _host_layout(inputs)
    S, B = L["S"], L["B"]
    assert B % N_CORES == 0
    shard = B // N_CORES
    TB = int(os.environ.get("BASS_LSTM_TB", "16"))
    if S % TB != 0:
        TB = 8 if S % 8 == 0 else 1

    nc, predicted_ns = _build_program(S, TB, shard, repeats=_repeats)
    _split_excess_waits(nc)
    if predicted_ns and os.environ.get("BASS_LSTM_VERBOSE"):
        print(f"[kernel] tile-predicted makespan: {predicted_ns} ns")

    in_maps = []
    for c in range(N_CORES):
        sl = slice(c * shard, (c + 1) * shard)
        in_maps.append(
            {
                "epsm": L["epsT"],
                "xc": np.ascontiguousarray(L["xc_all"][:, :, sl]),
                "mum": L["mu_m"],
                "rhom": L["rho_m"],
                "h0T": np.ascontiguousarray(L["h0T"][:, sl]),
                "c0T": np.ascontiguousarray(L["c0T"][:, sl]),
                "owm": L["ow_m"],
                "owr": L["ow_r"],
                "eow": L["e_ow"],
                "obm": L["ob_m"],
                "obr": L["ob_r"],
                "eob": L["e_ob"],
            }
        )

    return nc, in_maps, shard, predicted_ns


def kernel(**inputs):
    nc, in_maps, shard, _pred = prepare(**inputs)
    res = run_bass_kernel_spmd(nc, in_maps, list(range(N_CORES)), trace=False)
    out = np.concatenate(
        [res.results[c]["out"] for c in range(N_CORES)], axis=0
    ).astype(np.float32)
    return out


# revision 41
# speedup vs baseline: 1.4471x; 1.4471x over previous
"""Bayes-by-Backprop LSTM on 8 Trainium2 NeuronCores (Bass/Tile).

V3 strategy (data parallel, transposed recurrence, bf16):
  - Shard batch B=256 across 8 cores (32 rows each); eps/weights replicated.
  - Host-side layout only (transpose/reshape/concat + dtype casts, no math):
      * merged weight rows [x(64); bias(1); h(128)] = 193, columns (gate, h)
        gate order (i, f, ch, o); ch columns later doubled on device so all
        four gates go through one Sigmoid (tanh(x) = 2*sigmoid(2x)-1).
      * eps_w/eps_b merged the same way, laid out [193, S, 512] bf16.
      * x transposed to [64, S, B] bf16 with a ones-row (bias via matmul).
  - Device per step t (all transposed: batch is the matmul MOVING dim N=32):
      w1 = sig1*eps1[t] + mu1 ; w2 = sig2*eps2[t] + mu2   (DVE stt, bf16 4x)
      per gate gi: P[:,gi,:] = w2[:,gi]^T @ xcT_t + w1[:,gi]^T @ hT
        (sampled weights are the PE STATIONARY [K,128]; moving N=32)
      s = sigmoid(P) [128, 4*32] -> bf16
      cT' = f*cT + i*tanh-part ; hT' = o*tanh(cT')  (DVE/Pool, [128,32] ops)
    hT' [128, 32] bf16 feeds the next step's matmul directly - no transpose.
  - Final: out = h_S @ (out_mu + softplus(out_rho)*eps_out) + bias via ones row.
"""

import os
import sys
import numpy as np

for _p in ("/opt/trn_rl_repo",):
    if _p not in sys.path and os.path.isdir(_p):
        sys.path.append(_p)

import ml_dtypes  # noqa: E402
import concourse.bass as bass  # noqa: E402
import concourse.tile as tile  # noqa: E402
from concourse import mybir  # noqa: E402
from concourse.bass_utils import run_bass_kernel_spmd  # noqa: E402

F32 = mybir.dt.float32
BF16 = mybir.dt.bfloat16
AF = mybir.ActivationFunctionType
ALU = mybir.AluOpType
N_CORES = 8
GPERM = [0, 1, 2, 3]  # gate column order: i, f, ch, o

# The walrus bundled in this container rejects instructions carrying more
# than 2 semaphore-wait commands; Tile freely emits 3+. Split the excess
# onto same-engine NOPs inserted immediately before the instruction.
MAX_INST_WAITS = 1


def _split_excess_waits(nc, max_waits=MAX_INST_WAITS):
    blocks = []
    for f in nc.m.functions:
        for blk in f.blocks:
            blocks.append((blk, list(blk.instructions)))

    plans = {}  # id(inst) -> list of nop mybir instructions to insert before
    for blk, insts in blocks:
        for inst in insts:
            si = inst.sync_info
            if si is None:
                continue
            waits = list(si.on_wait)
            if len(waits) <= max_waits:
                continue
            ge = [w for w in waits if w.wait_mode == "sem-ge-imm"]
            other = [w for w in waits if w.wait_mode != "sem-ge-imm"]
            assert len(other) <= max_waits, (
                f"{inst.name}: {len(other)} non-ge waits, cannot split"
            )
            keep_n = max_waits - len(other)
            kept = other + (ge[len(ge) - keep_n :] if keep_n > 0 else [])
            excess = ge[: len(ge) - keep_n] if keep_n > 0 else ge
            eng = inst.engine
            nops = []
            for k in range(0, len(excess), max_waits):
                nop = nc.engines[eng].nop()
                nop.ins.sync_info = mybir.SyncInfo(
                    on_wait=list(excess[k : k + max_waits]), on_update=[]
                )
                nops.append(nop.ins)
            inst.sync_info = mybir.SyncInfo(
                on_wait=kept, on_update=list(si.on_update)
            )
            plans[id(inst)] = nops

    if not plans:
        return
    for blk, orig in blocks:
        new = []
        for inst in orig:
            new.extend(plans.get(id(inst), ()))
            new.append(inst)
        blk.instructions = new


def _build_program(S, TB, shard, repeats=1):
    """Emit the bass program for one core (SPMD across 8)."""
    nc = bass.Bass()
    d_eps = nc.declare_dram_parameter("epsm", [193, S, 512], BF16, isOutput=False)
    d_xc = nc.declare_dram_parameter("xc", [65, S, shard], BF16, isOutput=False)
    d_mu = nc.declare_dram_parameter("mum", [193, 512], F32, isOutput=False)
    d_rho = nc.declare_dram_parameter("rhom", [193, 512], F32, isOutput=False)
    d_h0T = nc.declare_dram_parameter("h0T", [128, shard], F32, isOutput=False)
    d_c0T = nc.declare_dram_parameter("c0T", [128, shard], F32, isOutput=False)
    d_owm = nc.declare_dram_parameter("owm", [128, 8], F32, isOutput=False)
    d_owr = nc.declare_dram_parameter("owr", [128, 8], F32, isOutput=False)
    d_eow = nc.declare_dram_parameter("eow", [128, 8], F32, isOutput=False)
    d_obm = nc.declare_dram_parameter("obm", [1, 8], F32, isOutput=False)
    d_obr = nc.declare_dram_parameter("obr", [1, 8], F32, isOutput=False)
    d_eob = nc.declare_dram_parameter("eob", [1, 8], F32, isOutput=False)
    d_out = nc.declare_dram_parameter("out", [shard, 8], F32, isOutput=True)

    IL = int(os.environ.get("BASS_LSTM_IL", "2"))  # batch interleave chains
    assert shard % IL == 0
    hsh = shard // IL

    from contextlib import ExitStack

    with tile.TileContext(nc) as tc, ExitStack() as ctx:
        singles = ctx.enter_context(tc.tile_pool(name="singles", bufs=1))
        _nb = 3 if TB <= 8 else 2
        pe1 = ctx.enter_context(tc.tile_pool(name="pe1", bufs=_nb))
        pe2 = ctx.enter_context(tc.tile_pool(name="pe2", bufs=_nb))
        px = ctx.enter_context(tc.tile_pool(name="px", bufs=_nb))
        pw1 = ctx.enter_context(tc.tile_pool(name="pw1", bufs=2))
        pw2 = ctx.enter_context(tc.tile_pool(name="pw2", bufs=2))
        psm = ctx.enter_context(tc.tile_pool(name="psm", bufs=12 * IL))
        phT = ctx.enter_context(tc.tile_pool(name="phT", bufs=3 * IL))
        psum_g = ctx.enter_context(
            tc.tile_pool(name="psum_g", bufs=4, space=bass.MemorySpace.PSUM)
        )
        psum_o = ctx.enter_context(
            tc.tile_pool(name="psum_o", bufs=1, space=bass.MemorySpace.PSUM)
        )

        # ---- constants in SBUF
        mu1f = singles.tile([128, 512], F32)
        mu2f = singles.tile([65, 512], F32)
        nc.gpsimd.dma_start(mu1f[:], d_mu[65:193, :])
        nc.gpsimd.dma_start(mu2f[:], d_mu[0:65, :])
        rho1 = singles.tile([128, 512], F32)
        rho2 = singles.tile([65, 512], F32)
        nc.gpsimd.dma_start(rho1[:], d_rho[65:193, :])
        nc.gpsimd.dma_start(rho2[:], d_rho[0:65, :])

        # softplus(x) = ln(1 + exp(x)) - ln+exp share one table set
        def softplus(out_ap, in_ap, tmp_ap):
            nc.scalar.activation(tmp_ap, in_ap, AF.Exp)
            nc.vector.tensor_scalar_add(tmp_ap, tmp_ap, 1.0)
            nc.scalar.activation(out_ap, tmp_ap, AF.Ln)

        sig1f = singles.tile([128, 512], F32)
        sig2f = singles.tile([65, 512], F32)
        spt1 = singles.tile([128, 512], F32)
        spt2 = singles.tile([65, 512], F32)
        softplus(sig1f[:], rho1[:], spt1[:])
        softplus(sig2f[:], rho2[:], spt2[:])

        # ch-gate columns (256:384) pre-doubled: tanh(x) = 2*sigmoid(2x)-1 so
        # all four gates go through one Sigmoid instruction.
        for ap in (sig1f[:, 256:384], sig2f[:, 256:384],
                   mu1f[:, 256:384], mu2f[:, 256:384]):
            nc.vector.tensor_scalar_mul(ap, ap, 2.0)

        sig1 = singles.tile([128, 512], BF16)
        sig2 = singles.tile([65, 512], BF16)
        mu1 = singles.tile([128, 512], BF16)
        mu2 = singles.tile([65, 512], BF16)
        nc.vector.tensor_copy(sig1[:], sig1f[:])
        nc.vector.tensor_copy(sig2[:], sig2f[:])
        nc.vector.tensor_copy(mu1[:], mu1f[:])
        nc.vector.tensor_copy(mu2[:], mu2f[:])

        ones1f = singles.tile([1, shard], F32)
        nc.gpsimd.memset(ones1f[:], 1.0)
        ones1 = singles.tile([1, shard], BF16)
        nc.vector.tensor_copy(ones1[:], ones1f[:])

        # ---- output projection weights (tiny, f32 math then bf16)
        owm = singles.tile([128, 8], F32)
        owr = singles.tile([128, 8], F32)
        eow = singles.tile([128, 8], F32)
        nc.gpsimd.dma_start(owm[:], d_owm[:])
        nc.gpsimd.dma_start(owr[:], d_owr[:])
        nc.gpsimd.dma_start(eow[:], d_eow[:])
        sow = singles.tile([128, 8], F32)
        sowt = singles.tile([128, 8], F32)
        softplus(sow[:], owr[:], sowt[:])
        wtmp = singles.tile([128, 8], F32)
        nc.vector.tensor_mul(wtmp[:], sow[:], eow[:])
        woutf = singles.tile([128, 8], F32)
        nc.vector.tensor_add(woutf[:], wtmp[:], owm[:])
        wout = singles.tile([128, 8], BF16)
        nc.vector.tensor_copy(wout[:], woutf[:])

        obm = singles.tile([1, 8], F32)
        obr = singles.tile([1, 8], F32)
        eob = singles.tile([1, 8], F32)
        nc.gpsimd.dma_start(obm[:], d_obm[:])
        nc.gpsimd.dma_start(obr[:], d_obr[:])
        nc.gpsimd.dma_start(eob[:], d_eob[:])
        sob = singles.tile([1, 8], F32)
        sobt = singles.tile([1, 8], F32)
        softplus(sob[:], obr[:], sobt[:])
        btmp = singles.tile([1, 8], F32)
        nc.vector.tensor_mul(btmp[:], sob[:], eob[:])
        boutf = singles.tile([1, 8], F32)
        nc.vector.tensor_add(boutf[:], btmp[:], obm[:])
        bout = singles.tile([1, 8], BF16)
        nc.vector.tensor_copy(bout[:], boutf[:])

        # ---- replicate sig across TB for bulk per-block weight multiplies
        sig1r = singles.tile([128, TB, 512], BF16)
        sig2r = singles.tile([65, TB, 512], BF16)
        for k in range(TB):
            nc.vector.tensor_copy(sig1r[:, k, :], sig1[:])
            nc.vector.tensor_copy(sig2r[:, k, :], sig2[:])

        # column split point between DVE and Pool for the weight multiplies
        C1 = int(os.environ.get("BASS_LSTM_C1", "384"))  # w1 (h-block) split
        C2 = int(os.environ.get("BASS_LSTM_C2", "256"))  # w2 (x-block) split

        # ---- state (per interleave chain): cT f32, hT bf16 (SBUF)
        cstall = singles.tile([128, IL, hsh], F32)
        h0f = singles.tile([128, shard], F32)
        nc.gpsimd.dma_start(h0f[:], d_h0T[:])
        assert S % TB == 0
        hT = [None] * IL
        for rep in range(repeats):
            for ch in range(IL):
                nc.gpsimd.dma_start(
                    cstall[:, ch, :], d_c0T[:, ch * hsh:(ch + 1) * hsh])
                h0b = phT.tile([128, hsh], BF16)
                nc.vector.tensor_copy(
                    h0b[:], h0f[:, ch * hsh:(ch + 1) * hsh])
                hT[ch] = h0b

            e1 = e2 = xb = w1 = w2 = None
            for t in range(S):
                tl = t % TB
                if tl == 0:
                    t0 = t
                    # DMA queues: keep the critical-cycle engines (ACT, Pool,
                    # PE) free. e1 -> SP; e2 split SP/DVE; xb -> SP.
                    e1 = pe1.tile([128, TB, 512], BF16)
                    nc.sync.dma_start(e1[:], d_eps[65:193, t0:t0 + TB, :])
                    e2 = pe2.tile([65, TB, 512], BF16)
                    nc.gpsimd.dma_start(e2[:], d_eps[0:65, t0:t0 + TB, :])
                    xb = px.tile([65, TB, shard], BF16)
                    nc.sync.dma_start(xb[:], d_xc[:, t0:t0 + TB, :])
                    # bulk sampled weights for the whole block; DVE and Pool
                    # are off the critical cycle. h-block fully folds mu so
                    # its mu matmul disappears (w1 = sig*eps + mu).
                    # bulk sampled-weight noise for the whole block on DVE
                    # (tensor_tensor: 2x mode on bf16); DVE is off-cycle.
                    w1 = pw1.tile([128, TB, 512], BF16)
                    nc.vector.tensor_mul(w1[:], e1[:], sig1r[:])
                    w2 = pw2.tile([65, TB, 512], BF16)
                    nc.vector.tensor_mul(w2[:], e2[:], sig2r[:])

                for ch in range(IL):
                    b0 = ch * hsh
                    g = psum_g.tile([128, 4, hsh], F32)
                    xlt = xb[:, tl, b0:b0 + hsh]
                    # one start->stop accumulation group per gate slice; a
                    # PSUM zero region admits only one pending group at a time
                    for gi in range(4):
                        c0_, c1_ = gi * 128, (gi + 1) * 128
                        nc.tensor.matmul(
                            g[:, gi, :], mu2[:, c0_:c1_], xlt,
                            start=True, stop=False)
                        nc.tensor.matmul(
                            g[:, gi, :], w2[:, tl, c0_:c1_], xlt,
                            start=False, stop=False)
                        nc.tensor.matmul(
                            g[:, gi, :], mu1[:, c0_:c1_], hT[ch][:],
                            start=False, stop=False)
                        nc.tensor.matmul(
                            g[:, gi, :], w1[:, tl, c0_:c1_], hT[ch][:],
                            start=False, stop=True)

                    s = psm.tile([128, 4, hsh], BF16)
                    nc.scalar.activation(s[:], g[:], AF.Sigmoid)

                    # i*tanh(g_ch)/2 = s_i*(s_ch'-0.5), s_ch' = sigmoid(2 g_ch)
                    # small recurrence ops consecutively on DVE: same-engine
                    # ordering avoids cross-engine hops on the critical cycle
                    v2 = psm.tile([128, hsh], BF16)
                    nc.vector.scalar_tensor_tensor(
                        v2[:], s[:, 2, :], -0.5, s[:, 0, :], ALU.add, ALU.mult)
                    fc = psm.tile([128, hsh], F32)
                    nc.gpsimd.tensor_mul(fc[:], s[:, 1, :], cstall[:, ch, :])
                    nc.vector.scalar_tensor_tensor(
                        cstall[:, ch, :], v2[:], 2.0, fc[:], ALU.mult, ALU.add)
                    th = psm.tile([128, hsh], BF16)
                    nc.scalar.activation(th[:], cstall[:, ch, :], AF.Tanh)
                    hnew = phT.tile([128, hsh], BF16)
                    nc.gpsimd.tensor_mul(hnew[:], s[:, 3, :], th[:])
                    hT[ch] = hnew

        # ---- output projection
        for ch in range(IL):
            b0 = ch * hsh
            ops = psum_o.tile([hsh, 8], F32, name=f"ops{ch}")
            nc.tensor.matmul(ops[:], hT[ch][:], wout[:],
                             start=True, stop=False)
            nc.tensor.matmul(ops[:], ones1[:, b0:b0 + hsh],
                             bout[:], start=False, stop=True)
            osb = psm.tile([hsh, 8], F32, name=f"osb{ch}")
            nc.vector.tensor_copy(osb[:], ops[:])
            nc.gpsimd.dma_start(d_out[b0:b0 + hsh, :], osb[:])

    predicted_ns = None
    try:
        ent = tc._perfetto_entries
        if ent:
            predicted_ns = int(max(max(e[1] or 0, e[2] or 0) for e in ent))
    except Exception:
        pass
    return nc, predicted_ns


def _host_layout(inputs):
    x = np.asarray(inputs["x"], np.float32)
    h0 = np.asarray(inputs["h0"], np.float32)
    c0 = np.asarray(inputs["c0"], np.float32)
    w_mu = np.asarray(inputs["w_mu"], np.float32)
    w_rho = np.asarray(inputs["w_rho"], np.float32)
    b_mu = np.asarray(inputs["b_mu"], np.float32)
    b_rho = np.asarray(inputs["b_rho"], np.float32)
    eps_w = np.asarray(inputs["eps_w"], np.float32)
    eps_b = np.asarray(inputs["eps_b"], np.float32)

    B, S, I = x.shape
    H = h0.shape[1]
    G = 4
    GH = G * H

    def merge_rows(w_g, b_g):  # w_g [G, I+H, H], b_g [G, H] -> [I+1+H, G*H]
        rows = np.transpose(w_g, (1, 0, 2)).reshape(I + H, GH)
        brow = b_g.reshape(1, GH)
        return np.concatenate([rows[:I], brow, rows[I:]], axis=0)

    mu_m = np.ascontiguousarray(merge_rows(w_mu[GPERM], b_mu[GPERM]))
    rho_m = np.ascontiguousarray(merge_rows(w_rho[GPERM], b_rho[GPERM]))

    ew = eps_w[:, GPERM]  # [S, G, I+H, H]
    eps_rows = np.transpose(ew, (0, 2, 1, 3)).reshape(S, I + H, GH)
    eb_row = eps_b[:, GPERM].reshape(S, 1, GH)
    eps_m = np.concatenate([eps_rows[:, :I], eb_row, eps_rows[:, I:]], axis=1)
    epsT = np.ascontiguousarray(
        np.transpose(eps_m, (1, 0, 2)).astype(ml_dtypes.bfloat16)
    )  # [193, S, GH] bf16

    xT = np.transpose(x, (2, 1, 0))  # [I, S, B]
    ones_row = np.ones((1, S, B), np.float32)
    xc_all = np.concatenate([xT, ones_row], axis=0).astype(
        ml_dtypes.bfloat16
    )  # [I+1, S, B] bf16

    h0T = np.ascontiguousarray(h0.T)  # [H, B]
    c0T = np.ascontiguousarray(c0.T)  # [H, B]

    ow_m = np.asarray(inputs["out_w_mu"], np.float32)
    ow_r = np.asarray(inputs["out_w_rho"], np.float32)
    e_ow = np.asarray(inputs["eps_out_w"], np.float32)
    ob_m = np.asarray(inputs["out_b_mu"], np.float32).reshape(1, -1)
    ob_r = np.asarray(inputs["out_b_rho"], np.float32).reshape(1, -1)
    e_ob = np.asarray(inputs["eps_out_b"], np.float32).reshape(1, -1)

    return dict(
        S=S, B=B, epsT=epsT, mu_m=mu_m, rho_m=rho_m, xc_all=xc_all,
        h0T=h0T, c0T=c0T, ow_m=ow_m, ow_r=ow_r, e_ow=e_ow,
        ob_m=ob_m, ob_r=ob_r, e_ob=e_ob,
    )


def prepare(_repeats=1, **inputs):
    """Build the bass program + per-core input maps."""
    L = _host_layout(inputs)
    S, B = L["S"], L["B"]
    assert B % N_CORES == 0
    shard = B // N_CORES
    TB = int(os.environ.get("BASS_LSTM_TB", "16"))
    if S % TB != 0:
        TB = 8 if S % 8 == 0 else 1

    nc, predicted_ns = _build_program(S, TB, shard, repeats=_repeats)
    _split_excess_waits(nc)
    if predicted_ns and os.environ.get("BASS_LSTM_VERBOSE"):
        print(f"[kernel] tile-predicted makespan: {predicted_ns} ns")

    in_maps = []
    for c in range(N_CORES):
        sl = slice(c * shard, (c + 1) * shard)
        in_maps.append(
            {
                "epsm": L["epsT"],
                "xc": np.ascontiguousarray(L["xc_all"][:, :, sl]),
                "mum": L["mu_m"],
                "rhom": L["rho_m"],
                "h0T": np.ascontiguousarray(L["h0T"][:, sl]),
                "c0T": np.ascontiguousarray(L["c0T"][:, sl]),
                "owm": L["ow_m"],
                "owr": L["ow_r"],
                "eow": L["e_ow"],
                "obm": L["ob_m"],
                "obr": L["ob_r"],
                "eob": L["e_ob"],
            }
        )

    return nc, in_maps, shard, predicted_ns


def kernel(**inputs):
    nc, in_maps, shard, _pred = prepare(**inputs)
    res = run_bass_kernel_spmd(nc, in_maps, list(range(N_CORES)), trace=False)
    out = np.concatenate(
        [res.results[c]["out"] for c in range(N_CORES)], axis=0
    ).astype(np.float32)
    return out
